# revision 20
# baseline (speedup 1.0000x reference)
"""Beran survival estimator (KDE-weighted Kaplan-Meier) on Trainium2.

Full inputs:
  x_in     [B, n, d] f32
  delta_in [B, n]    f32
  x_p      [B, d]    f32
  bandwidth [1]      f32

Outputs: (surv_func [B, n], surv_steps [B, n-1]) both f32.

Sharding: pure data-parallel over B across 8 NeuronCores (B_loc = B/8 per core).

Per-core layout: sample index n is mapped as n = n_hi*CH + f with
partition p = b*NH + n_hi  (NH = 128/B_loc chunks per batch row,
CH = n/NH elements per chunk along the free axis).  Every per-[B,n]
tensor is then a single [128, CH] tile whose partition p holds a
contiguous CH-slice of one batch row -- DMAs are fully contiguous and
all elementwise work is full-width.

Cumulative sums over n: one tensor_tensor_scan along f (all 128
partitions at once) + a cross-partition exclusive prefix of the chunk
totals within each batch's NH-partition group, done with a masked
strictly-triangular-ones matmul on the (otherwise idle) tensor engine,
applied as a per-partition scalar.

sqrt(m2) is computed as exp(0.5*ln(m2)) so that every ScalarE
transcendental (Square, Ln, Exp) lives in the natural_log_exp table set:
zero ACT table reloads in the whole kernel.
"""

import numpy as np

_CACHE: dict = {}

M_CORES = 8
P = 128


def _build(B_loc: int, n: int, d: int, F: int, act_per_block: int = 5):
    """Emit + compile the per-core Bass program."""
    from contextlib import ExitStack

    import concourse.bacc as bacc
    import concourse.tile as tile
    from concourse import mybir

    Fn = mybir.ActivationFunctionType
    A = mybir.AluOpType
    f32 = mybir.dt.float32

    NH = P // B_loc          # n-chunks per batch row
    CH = n // NH             # chunk length (free axis)
    NB = CH // F             # f-blocks
    assert NH * B_loc == P and CH * NH == n and NB * F == CH

    # jnp.isclose(a, 1.0) threshold in f32
    THR = float(np.float32(np.float32(1e-8) + np.float32(1e-5) * np.float32(1.0)))
    LO = float(np.float32(1.0) - np.float32(THR))
    HI = float(np.float32(1.0) + np.float32(THR))
    EPS_SUM = 1e-13

    nc = bacc.Bacc("TRN2", target_bir_lowering=False, debug=False)

    x_d = nc.dram_tensor("x", [B_loc, n, d], f32, kind="ExternalInput").ap()
    xpm_d = nc.dram_tensor("xpm", [P, d], f32, kind="ExternalInput").ap()
    delta_d = nc.dram_tensor("delta", [B_loc, n], f32, kind="ExternalInput").ap()
    nlb_d = nc.dram_tensor("nlb", [P, 1], f32, kind="ExternalInput").ap()
    ub_d = nc.dram_tensor("ubm", [P, P], f32, kind="ExternalInput").ap()
    bb_d = nc.dram_tensor("bbm", [P, P], f32, kind="ExternalInput").ap()
    sb_d = nc.dram_tensor("sbm", [P, P], f32, kind="ExternalInput").ap()
    surv_d = nc.dram_tensor("surv", [B_loc, n], f32, kind="ExternalOutput").ap()
    steps_d = nc.dram_tensor("steps", [B_loc, n], f32, kind="ExternalOutput").ap()

    xr = x_d.rearrange("b (nh f) dd -> (b nh) f dd", nh=NH)
    delta_r = delta_d.rearrange("b (nh f) -> (b nh) f", nh=NH)
    surv_r = surv_d.rearrange("b (nh f) -> (b nh) f", nh=NH)
    steps_r = steps_d.rearrange("b (nh f) -> (b nh) f", nh=NH)

    with tile.TileContext(nc) as tc, ExitStack() as ctx:
        consts = ctx.enter_context(tc.tile_pool(name="consts", bufs=1))
        xpool = ctx.enter_context(tc.tile_pool(name="xpool", bufs=6))
        dpool = ctx.enter_context(tc.tile_pool(name="dpool", bufs=3))
        spool = ctx.enter_context(tc.tile_pool(name="spool", bufs=2))
        qpool = ctx.enter_context(tc.tile_pool(name="qpool", bufs=2))
        big = ctx.enter_context(tc.tile_pool(name="big", bufs=1))
        psum = ctx.enter_context(tc.tile_pool(name="psum", bufs=1, space="PSUM"))
        pspool = ctx.enter_context(tc.tile_pool(name="pspool", bufs=2, space="PSUM"))

        m2 = big.tile([P, CH], f32)
        xpm = consts.tile([P, d], f32)
        nc.scalar.dma_start(xpm[:], xpm_d)

        # ---- phase 1: m2[p, f] = ||x[b(p), n(p,f)] - xp[b(p)]||^2 --------
        for tt in range(NB):
            f0 = tt * F
            xt = xpool.tile([P, F * d], f32)
            nc.sync.dma_start(
                xt[:].rearrange("p (t dd) -> p t dd", t=F),
                xr[:, f0:f0 + F, :],
            )
            diff = dpool.tile([P, F * d], f32)
            nc.vector.tensor_sub(
                diff[:].rearrange("p (t dd) -> p t dd", t=F),
                xt[:].rearrange("p (t dd) -> p t dd", t=F),
                xpm[:].unsqueeze(1).broadcast_to([P, F, d]),
            )
            d3 = diff[:].rearrange("p (t dd) -> p t dd", t=F)
            nact = act_per_block + (1 if tt % 4 == 0 else 0)
            for j in range(F):
                col = m2[:, f0 + j:f0 + j + 1]
                if j < nact:
                    scr = spool.tile([P, d], f32)
                    nc.scalar.activation(
                        scr[:], d3[:, j, :], Fn.Square, accum_out=col
                    )
                else:
                    scr2 = qpool.tile([P, d], f32)
                    nc.vector.scalar_tensor_tensor(
                        out=scr2[:], in0=d3[:, j, :], scalar=1.0,
                        in1=d3[:, j, :],
                        op0=A.mult, op1=A.mult, accum_out=col,
                    )

        # constants + delta loaded on a different queue late (not needed
        # until phase 2)
        nlb = consts.tile([P, 1], f32)
        nc.scalar.dma_start(nlb[:], nlb_d)
        ub = consts.tile([P, P], f32)
        nc.scalar.dma_start(ub[:], ub_d)
        bb = consts.tile([P, P], f32)
        nc.scalar.dma_start(bb[:], bb_d)
        sb = consts.tile([P, P], f32)
        nc.scalar.dma_start(sb[:], sb_d)
        delta_sb = big.tile([P, CH], f32)
        nc.scalar.dma_start(delta_sb[:], delta_r)

        # ---- phase 2 ------------------------------------------------------
        # metric/bw = exp(0.5*ln(m2) - ln(bw)); w = exp(-metric/bw)
        u = big.tile([P, CH], f32)
        nc.scalar.activation(u[:], m2[:], Fn.Ln)
        v = big.tile([P, CH], f32)
        nc.scalar.activation(v[:], u[:], Fn.Exp, bias=nlb[:, 0:1], scale=0.5)
        w = big.tile([P, CH], f32)
        nc.scalar.activation(w[:], v[:], Fn.Exp, scale=-1.0)

        cum = big.tile([P, CH], f32)
        nc.vector.tensor_tensor_scan(
            cum[:], w[:], w[:], 0.0, op0=A.add, op1=A.bypass
        )

        totals = cum[:, CH - 1:CH]
        offs_ps = psum.tile([P, 1], f32)
        nc.tensor.matmul(offs_ps[:], ub[:], totals, start=True, stop=True)
        sbc_ps = psum.tile([P, 1], f32)
        nc.tensor.matmul(sbc_ps[:], bb[:], totals, start=True, stop=True)

        offs = big.tile([P, 1], f32)
        nc.vector.tensor_copy(offs[:], offs_ps[:])
        ssafe = big.tile([P, 1], f32)
        nc.vector.tensor_scalar(ssafe[:], sbc_ps[:], EPS_SUM, None, op0=A.max)
        inv_s = big.tile([P, 1], f32)
        nc.vector.reciprocal(inv_s[:], ssafe[:])
        smask = big.tile([P, 1], f32)
        nc.vector.tensor_scalar(smask[:], sbc_ps[:], EPS_SUM, None, op0=A.is_ge)
        nc.vector.tensor_mul(inv_s[:], inv_s[:], smask[:])

        # cumn = (cum + offs) * inv_s ; shfn = (cum - w + offs) * inv_s
        cumn = big.tile([P, CH], f32)
        nc.vector.tensor_scalar(
            cumn[:], cum[:], offs[:, 0:1], inv_s[:, 0:1], op0=A.add, op1=A.mult
        )
        shfr = big.tile([P, CH], f32)
        nc.vector.tensor_sub(shfr[:], cum[:], w[:])
        shfn = big.tile([P, CH], f32)
        nc.vector.tensor_scalar(
            shfn[:], shfr[:], offs[:, 0:1], inv_s[:, 0:1], op0=A.add, op1=A.mult
        )

        # bad = isclose(cumn,1)|isclose(shfn,1);  good = !bad
        t1 = big.tile([P, CH], f32)
        nc.vector.tensor_scalar(t1[:], cumn[:], LO, None, op0=A.is_ge)
        bad1 = big.tile([P, CH], f32)
        nc.vector.tensor_scalar(bad1[:], cumn[:], HI, None, op0=A.is_le)
        nc.vector.tensor_mul(bad1[:], bad1[:], t1[:])
        t2 = big.tile([P, CH], f32)
        nc.vector.tensor_scalar(t2[:], shfn[:], LO, None, op0=A.is_ge)
        bad2 = big.tile([P, CH], f32)
        nc.vector.tensor_scalar(bad2[:], shfn[:], HI, None, op0=A.is_le)
        nc.vector.tensor_mul(bad2[:], bad2[:], t2[:])
        good = big.tile([P, CH], f32)
        nc.vector.tensor_tensor(good[:], bad1[:], bad2[:], op=A.logical_or)
        nc.vector.tensor_scalar(good[:], good[:], 0.0, None, op0=A.is_equal)
        cumz = big.tile([P, CH], f32)
        nc.vector.tensor_mul(cumz[:], cumn[:], good[:])
        shfz = big.tile([P, CH], f32)
        nc.vector.tensor_mul(shfz[:], shfn[:], good[:])

        # xi = ln(1 - shfz) - ln(1 - cumz), then * delta
        l1 = big.tile([P, CH], f32)
        nc.scalar.activation(l1[:], shfz[:], Fn.Ln, bias=1.0, scale=-1.0)
        l2 = big.tile([P, CH], f32)
        nc.scalar.activation(l2[:], cumz[:], Fn.Ln, bias=1.0, scale=-1.0)
        xi = big.tile([P, CH], f32)
        nc.vector.tensor_sub(xi[:], l1[:], l2[:])
        nc.vector.tensor_mul(xi[:], xi[:], delta_sb[:])

        # hazards cumsum + cross-chunk offsets; surv = exp(-(hc + offs2))
        hc = big.tile([P, CH], f32)
        nc.vector.tensor_tensor_scan(
            hc[:], xi[:], xi[:], 0.0, op0=A.add, op1=A.bypass
        )
        offs2_ps = psum.tile([P, 1], f32)
        nc.tensor.matmul(offs2_ps[:], ub[:], hc[:, CH - 1:CH], start=True, stop=True)
        noffs2 = big.tile([P, 1], f32)
        nc.vector.tensor_scalar(noffs2[:], offs2_ps[:], -1.0, None, op0=A.mult)
        surv = big.tile([P, CH], f32)
        nc.scalar.activation(surv[:], hc[:], Fn.Exp, bias=noffs2[:, 0:1], scale=-1.0)

        nc.sync.dma_start(surv_r, surv[:])

        # steps[n] = surv[n] - surv[n+1]
        steps = big.tile([P, CH], f32)
        nc.vector.tensor_sub(steps[:, 0:CH - 1], surv[:, 0:CH - 1], surv[:, 1:CH])
        shp_ps = psum.tile([P, 1], f32)
        nc.tensor.matmul(shp_ps[:], sb[:], surv[:, 0:1], start=True, stop=True)
        nc.vector.tensor_sub(steps[:, CH - 1:CH], surv[:, CH - 1:CH], shp_ps[:])

        nc.sync.dma_start(steps_r, steps[:])

    nc.compile()
    return nc


def _get_program(B_loc: int, n: int, d: int, F: int = 8, act_per_block: int = 5):
    key = (B_loc, n, d, F, act_per_block)
    if key not in _CACHE:
        _CACHE[key] = _build(B_loc, n, d, F, act_per_block)
    return _CACHE[key]


def _make_consts(bandwidth, B_loc):
    NH = P // B_loc
    bw = float(np.asarray(bandwidth).reshape(-1)[0])
    bwc = min(max(bw, 0.1), 10.0)
    nlb = np.full((P, 1), -np.log(np.float32(bwc)), dtype=np.float32)
    k = np.arange(P)
    same = (k[:, None] // NH) == (k[None, :] // NH)
    ub = (same & (k[:, None] < k[None, :])).astype(np.float32)
    bb = same.astype(np.float32)
    sb = (same & (k[:, None] == k[None, :] + 1)).astype(np.float32)
    return nlb, ub, bb, sb


def make_in_maps(x_in, delta_in, x_p, bandwidth, n_cores=M_CORES):
    B, n, d = x_in.shape
    B_loc = B // n_cores
    NH = P // B_loc
    nlb, ub, bb, sb = _make_consts(bandwidth, B_loc)
    in_maps = []
    for i in range(n_cores):
        sl = slice(i * B_loc, (i + 1) * B_loc)
        xpm = np.ascontiguousarray(
            np.repeat(np.asarray(x_p[sl], np.float32), NH, axis=0)
        )
        in_maps.append({
            "x": np.ascontiguousarray(np.asarray(x_in[sl], np.float32)),
            "xpm": xpm,
            "delta": np.ascontiguousarray(np.asarray(delta_in[sl], np.float32)),
            "nlb": nlb,
            "ubm": ub,
            "bbm": bb,
            "sbm": sb,
        })
    return in_maps


def kernel(x_in, delta_in, x_p, bandwidth):
    from concourse.bass_utils import run_bass_kernel_spmd

    x_in = np.asarray(x_in)
    B, n, d = x_in.shape
    B_loc = B // M_CORES
    nc = _get_program(B_loc, n, d)
    in_maps = make_in_maps(x_in, delta_in, x_p, bandwidth)
    res = run_bass_kernel_spmd(nc, in_maps, list(range(M_CORES)))
    surv = np.concatenate([r["surv"] for r in res.results], axis=0)
    steps = np.concatenate([r["steps"] for r in res.results], axis=0)[:, : n - 1]
    return surv.astype(np.float32, copy=False), steps.astype(np.float32, copy=False)
